# revision 1
# baseline (speedup 1.0000x reference)
"""MultiHeadAttention Trainium2 kernel.

Strategy: data-parallel, one batch element per NeuronCore (B=8 == n_cores).
Per core (batch b):
  - Host pre-transposes x_Q/x_K/x_V[b] -> xT [D, S] (bf16) and weights -> W^T (bf16),
    so every matmul has its contraction dim on partitions.
  - Projections on PE: Q^T/K^T in [hd, s] layout (head pairs stacked on 128
    partitions), V in natural [s, hd] layout grouped per head with an extra
    ones column (gives the softmax denominator for free during P@V).
  - scores^T [k, q] per (head, k-tile): QK^T with K=64 contraction; exp on
    ScalarE reading PSUM directly, applying scale=1/sqrt(64) and the key
    padding mask as a per-partition bias (-1e9 -> exp underflows to exact 0).
  - P@V with lhsT = P^T tile (M=128 queries) -> ctx in natural [q, hd] layout
    with denominator column; normalization is a per-partition reciprocal +
    broadcast multiply on VectorE (fused with the PSUM->SBUF copy).
  - ctx transposed via DMA xbar (bf16) for the output projection; final
    out = ctx @ Wo^T + bo in fp32.
Masked key tiles are computed but contribute exactly 0 (uniform SPMD program,
robust to any src_batch_lens values).
"""

import numpy as np
import ml_dtypes

import concourse.bass as bass  # noqa: F401
import concourse.tile as tile
from concourse import bacc, mybir
from concourse._compat import get_trn_type
from concourse.bass_utils import run_bass_kernel_spmd

B, S, D = 8, 2048, 512
H, DH = 8, 64
P = 128
NDT = D // P      # 4 tiles over the model/hd dim
NST = S // P      # 16 tiles over sequence (queries)
NKT = S // P      # 16 tiles over keys
F32 = mybir.dt.float32
BF16 = mybir.dt.bfloat16
NEG = -1.0e9

# stash for test.py introspection
last_results = None


DEBUG = False


def _build_program():
    nc = bacc.Bacc(get_trn_type() or "TRN2", target_bir_lowering=False)

    xqT_d = nc.dram_tensor("xqT", (P, NDT, S), BF16, kind="ExternalInput")
    xkT_d = nc.dram_tensor("xkT", (P, NDT, S), BF16, kind="ExternalInput")
    xvT_d = nc.dram_tensor("xvT", (P, NDT, S), BF16, kind="ExternalInput")
    wqT_d = nc.dram_tensor("wqT", (P, NDT, D), BF16, kind="ExternalInput")
    wkT_d = nc.dram_tensor("wkT", (P, NDT, D), BF16, kind="ExternalInput")
    wvT_d = nc.dram_tensor("wvT", (P, NDT, D), BF16, kind="ExternalInput")
    woT_d = nc.dram_tensor("woT", (P, NDT, D), BF16, kind="ExternalInput")
    bqT_d = nc.dram_tensor("bqT", (P, NDT), F32, kind="ExternalInput")
    bkT_d = nc.dram_tensor("bkT", (P, NDT), F32, kind="ExternalInput")
    bv_d = nc.dram_tensor("bvb", (P, D), F32, kind="ExternalInput")
    bo_d = nc.dram_tensor("bob", (P, D), F32, kind="ExternalInput")
    mask_d = nc.dram_tensor("mask", (P, NKT), F32, kind="ExternalInput")
    out_d = nc.dram_tensor("out", (P, NST, D), F32, kind="ExternalOutput")
    if DEBUG:
        qdbg_d = nc.dram_tensor("qdbg", (P, NDT, S), BF16, kind="ExternalOutput")
        kdbg_d = nc.dram_tensor("kdbg", (P, NDT, S), BF16, kind="ExternalOutput")
        vdbg_d = nc.dram_tensor("vdbg", (P, NST, H, DH + 1), BF16, kind="ExternalOutput")
        ctdbg_d = nc.dram_tensor("ctdbg", (P, NDT, S), BF16, kind="ExternalOutput")
        pdbg_d = nc.dram_tensor("pdbg", (P, NKT, 1024), BF16, kind="ExternalOutput")

    Exp = mybir.ActivationFunctionType.Exp
    MUL = mybir.AluOpType.mult
    ADD = mybir.AluOpType.add

    with tile.TileContext(nc) as tc:
        with tc.tile_pool(name="persist", bufs=1) as pp:
            wo_sb = pp.tile([P, NDT, D], BF16, tag="wo")
            nc.sync.dma_start(wo_sb[:], woT_d[:])
            mask_sb = pp.tile([P, NKT], F32, tag="mask")
            nc.sync.dma_start(mask_sb[:], mask_d[:])
            bqT_sb = pp.tile([P, NDT], F32, tag="bqT")
            nc.sync.dma_start(bqT_sb[:], bqT_d[:])
            bkT_sb = pp.tile([P, NDT], F32, tag="bkT")
            nc.sync.dma_start(bkT_sb[:], bkT_d[:])
            bv_sb = pp.tile([P, D], F32, tag="bv")
            nc.sync.dma_start(bv_sb[:], bv_d[:])
            bo_sb = pp.tile([P, D], F32, tag="bo")
            nc.sync.dma_start(bo_sb[:], bo_d[:])

            qT_sb = pp.tile([P, NDT, S], BF16, tag="qT")
            kT_sb = pp.tile([P, NDT, S], BF16, tag="kT")
            v_sb = pp.tile([P, NST, H, DH + 1], BF16, tag="v")
            ctxT_sb = pp.tile([P, NDT, S], BF16, tag="ctxT")

            # ones column for the denominator trick
            nc.vector.memset(v_sb[:, :, :, DH : DH + 1], 1.0)
            # e64: selects the denominator row (partition 64) in the
            # partition-broadcast matmul
            e64_sb = pp.tile([DH + 1, DH], BF16, tag="e64")
            nc.vector.memset(e64_sb[:], 0.0)
            nc.vector.memset(e64_sb[DH : DH + 1, :], 1.0)

            # ---- Shared PSUM pools (exactly 8 banks total) ----
            _scp_cm = tc.tile_pool(name="scps", bufs=2, space="PSUM")
            _cxp_cm = tc.tile_pool(name="cxps", bufs=2, space="PSUM")
            scp = _scp_cm.__enter__()
            cxp = _cxp_cm.__enter__()

            # ---- Phase 1: input loads + first projection tiles ----
            _xp_cm = tc.tile_pool(name="xin", bufs=1)
            xp = _xp_cm.__enter__()
            xq_sb = xp.tile([P, NDT, S], BF16, tag="xq")
            nc.sync.dma_start(xq_sb[:, :, 0:1024], xqT_d[:, :, 0:1024])
            nc.sync.dma_start(xq_sb[:, :, 1024:2048], xqT_d[:, :, 1024:2048])
            xk_sb = xp.tile([P, NDT, S], BF16, tag="xk")
            nc.sync.dma_start(xk_sb[:, :, 0:1024], xkT_d[:, :, 0:1024])
            nc.sync.dma_start(xk_sb[:, :, 1024:2048], xkT_d[:, :, 1024:2048])
            xv_sb = xp.tile([P, NDT, S], BF16, tag="xv")
            nc.sync.dma_start(xv_sb[:, :, 0:1024], xvT_d[:, :, 0:1024])
            nc.sync.dma_start(xv_sb[:, :, 1024:2048], xvT_d[:, :, 1024:2048])
            wq_sb = xp.tile([P, NDT, D], BF16, tag="wq")
            nc.sync.dma_start(wq_sb[:], wqT_d[:])
            wk_sb = xp.tile([P, NDT, D], BF16, tag="wk")
            nc.sync.dma_start(wk_sb[:], wkT_d[:])
            wv_sb = xp.tile([P, NDT, D], BF16, tag="wv")
            nc.sync.dma_start(wv_sb[:], wvT_d[:])

            def qk_proj_group(w_sb, x_sb, o_sb, b_sb, mt, half, pool, tag):
                # one [128, 1024] chunk of Q^T or K^T: out^T = (W^T).T @ x^T
                ps = pool.tile([P, 2, 512], F32, tag=tag, name="pj")
                for qc in range(2):
                    for kt in range(NDT):
                        nc.tensor.matmul(
                            ps[:, qc, :],
                            lhsT=w_sb[:, kt, mt * P : (mt + 1) * P],
                            rhs=x_sb[:, kt, half * 1024 + qc * 512 : half * 1024 + (qc + 1) * 512],
                            start=(kt == 0),
                            stop=(kt == NDT - 1),
                        )
                nc.vector.tensor_scalar_add(
                    o_sb[:, mt, half * 1024 : (half + 1) * 1024],
                    ps[:].rearrange("p a b -> p (a b)"),
                    b_sb[:, mt : mt + 1],
                )

            def v_proj_group(st, pool, tag):
                # V natural [s, hd] head-grouped with bias
                psv = pool.tile([P, 2, 512], F32, tag=tag, name="pv")
                for kt in range(NDT):
                    nc.tensor.matmul(
                        psv[:, 0, :],
                        lhsT=xv_sb[:, kt, st * P : (st + 1) * P],
                        rhs=wv_sb[:, kt, :],
                        start=(kt == 0),
                        stop=(kt == NDT - 1),
                    )
                nc.vector.tensor_tensor(
                    out=v_sb[:, st, :, 0:DH],
                    in0=psv[:, 0, :].rearrange("p (h d) -> p h d", h=H),
                    in1=bv_sb[:].rearrange("p (h d) -> p h d", h=H),
                    op=ADD,
                )

            # head-pair 0 (Mtile 0) of Q^T and K^T up front; the rest is
            # interleaved into the attention head loops below
            for half in range(2):
                qk_proj_group(wq_sb, xq_sb, qT_sb, bqT_sb, 0, half, scp, "sc")
                qk_proj_group(wk_sb, xk_sb, kT_sb, bkT_sb, 0, half, scp, "sc")
            # remaining projection groups, consumed inside the attention loops
            projq = []
            for mt in range(1, NDT):
                for half in range(2):
                    projq.append((wq_sb, xq_sb, qT_sb, bqT_sb, mt, half))
                    projq.append((wk_sb, xk_sb, kT_sb, bkT_sb, mt, half))

            # ---- Phase 2+3: attention, out-projection per q-half ----
            with tc.tile_pool(name="ptp", bufs=8) as ptp, \
                 tc.tile_pool(name="cup", bufs=4) as cup, \
                 tc.tile_pool(name="rrp", bufs=3) as rrp, \
                 tc.tile_pool(name="obp", bufs=4) as obp:
                def oproj_group(st, pool, tag):
                    pso = pool.tile([P, 2, 512], F32, tag=tag, name="pso")
                    for kt in range(NDT):
                        nc.tensor.matmul(
                            pso[:, 0, :],
                            lhsT=ctxT_sb[:, kt, st * P : (st + 1) * P],
                            rhs=wo_sb[:, kt, :],
                            start=(kt == 0),
                            stop=(kt == NDT - 1),
                        )
                    ot = obp.tile([P, D], F32, tag="ot")
                    nc.vector.tensor_tensor(out=ot[:], in0=pso[:, 0, :], in1=bo_sb[:], op=ADD)
                    nc.sync.dma_start(out_d[:, st, :], ot[:])

                oprojq = []
                pending_flush = [None]

                def run_pending():
                    if pending_flush[0] is not None:
                        pending_flush[0]()
                        pending_flush[0] = None

                for qh in range(2):
                    q0 = qh * 1024
                    for h in range(H):
                        pbase = (h % 2) * 64
                        hm = h // 2
                        # ctx^T accumulator rows 0..63 = head dims, row 64 =
                        # softmax denominator (ones column of V')
                        cxt = cxp.tile([P, 2, 512], F32, tag="cx")

                        def pv_step(t, pt):
                            for qc in range(2):
                                nc.tensor.matmul(
                                    cxt[0 : DH + 1, qc, :],
                                    lhsT=v_sb[:, t, h, :],
                                    rhs=pt[:, qc * 512 : (qc + 1) * 512],
                                    start=(t == 0),
                                    stop=(t == NKT - 1),
                                )

                        # software pipeline: PV for tile t-1 is emitted after
                        # QK/exp for tile t, so the PE never stalls behind ACT
                        prev = None
                        for t in range(NKT):
                            if qh == 0 and h == 0:
                                # V tile t is produced just in time for PV
                                v_proj_group(t, cxp, "cx")
                            pop_proj = (h == 1 and t % 4 == 2) or (
                                2 <= h <= 5 and t % 8 == 2
                            )
                            if qh == 0 and pop_proj and projq:
                                g = projq.pop(0)
                                qk_proj_group(*g, cxp, "cx")
                            if qh == 1 and t % 4 == 3 and oprojq:
                                oproj_group(oprojq.pop(0), cxp, "cx")
                            if t == 1:
                                # previous head's flush, deferred so its PE ops
                                # don't stall the stream at the head boundary
                                run_pending()
                            sc = scp.tile([P, 2, 512], F32, tag="sc")
                            for qc in range(2):
                                nc.tensor.matmul(
                                    sc[:, qc, :],
                                    lhsT=kT_sb[pbase : pbase + 64, hm, t * P : (t + 1) * P],
                                    rhs=qT_sb[pbase : pbase + 64, hm, q0 + qc * 512 : q0 + (qc + 1) * 512],
                                    start=True,
                                    stop=True,
                                )
                            pt = ptp.tile([P, 1024], BF16, tag="pt")
                            nc.scalar.activation(
                                pt[:],
                                sc[:].rearrange("p a b -> p (a b)"),
                                Exp,
                                bias=mask_sb[:, t : t + 1],
                                scale=0.125,
                            )
                            if DEBUG and h == 0 and qh == 0:
                                nc.sync.dma_start(pdbg_d[:, t, :], pt[:])
                            if prev is not None:
                                pv_step(*prev)
                            prev = (t, pt)
                        pv_step(*prev)
                        # flush: copy to SBUF, broadcast denom via PE, recip,
                        # normalize into ctx^T. The SBUF copy is emitted now
                        # (frees the cx slot); the PE/recip part is deferred
                        # into the next head's loop.
                        cu = cup.tile([DH + 1, 2, 512], BF16, tag="cu")
                        nc.vector.tensor_copy(cu[:], cxt[0 : DH + 1])

                        def flush(cu=cu, pbase=pbase, hm=hm, q0=q0):
                            rb = cxp.tile([P, 2, 512], F32, tag="cx", name="rb")
                            for qc in range(2):
                                nc.tensor.matmul(
                                    rb[0:DH, qc, :],
                                    lhsT=e64_sb[:],
                                    rhs=cu[:, qc, :],
                                    start=True,
                                    stop=True,
                                )
                            rc = rrp.tile([DH, 2, 512], F32, tag="rc")
                            nc.vector.reciprocal(rc[:], rb[0:DH])
                            nc.vector.tensor_tensor(
                                out=ctxT_sb[pbase : pbase + 64, hm, q0 : q0 + 1024],
                                in0=cu[0:DH].rearrange("p a b -> p (a b)"),
                                in1=rc[:].rearrange("p a b -> p (a b)"),
                                op=MUL,
                            )

                        run_pending()
                        pending_flush[0] = flush

                    # output projection: qh0's s-tiles are queued and
                    # interleaved into qh1's head loops; qh1's form the tail
                    run_pending()
                    if qh == 0:
                        oprojq.extend(range(0, 8))
                    else:
                        for st in oprojq:
                            oproj_group(st, scp, "sc")
                        oprojq = []
                        for st in range(8, 16):
                            oproj_group(st, scp, "sc")

            _xp_cm.__exit__(None, None, None)
            _cxp_cm.__exit__(None, None, None)
            _scp_cm.__exit__(None, None, None)

            if DEBUG:
                nc.sync.dma_start(qdbg_d[:], qT_sb[:])
                nc.sync.dma_start(kdbg_d[:], kT_sb[:])
                nc.sync.dma_start(vdbg_d[:], v_sb[:])
                nc.sync.dma_start(ctdbg_d[:], ctxT_sb[:])

    nc.compile()
    return nc


_program_cache = None


def _get_program():
    global _program_cache
    if _program_cache is None:
        _program_cache = _build_program()
    return _program_cache


def _to_bf16_T_tiled(x):
    # [S, D] fp32 -> x^T [D, S] -> [P, NDT, S] bf16 with d = dt*128 + p
    xt = np.ascontiguousarray(x.T.astype(ml_dtypes.bfloat16))
    return np.ascontiguousarray(xt.reshape(NDT, P, S).transpose(1, 0, 2))


def _w_T_tiled(w):
    # torch Linear weight [out, in] -> W^T [in, out] -> [P, NDT, out] bf16
    wt = np.ascontiguousarray(w.T.astype(ml_dtypes.bfloat16))
    return np.ascontiguousarray(wt.reshape(NDT, P, w.shape[0]).transpose(1, 0, 2))


def kernel(**inputs):
    global last_results
    x_Q = np.asarray(inputs["x_Q"], dtype=np.float32)
    x_K = np.asarray(inputs["x_K"], dtype=np.float32)
    x_V = np.asarray(inputs["x_V"], dtype=np.float32)
    Wq = np.asarray(inputs["Wq"], dtype=np.float32)
    Wk = np.asarray(inputs["Wk"], dtype=np.float32)
    Wv = np.asarray(inputs["Wv"], dtype=np.float32)
    Wo = np.asarray(inputs["Wo"], dtype=np.float32)
    bq = np.asarray(inputs["bq"], dtype=np.float32)
    bk = np.asarray(inputs["bk"], dtype=np.float32)
    bv = np.asarray(inputs["bv"], dtype=np.float32)
    bo = np.asarray(inputs["bo"], dtype=np.float32)
    lens = np.asarray(inputs["src_batch_lens"]).astype(np.int64)

    nc = _get_program()

    wqT = _w_T_tiled(Wq)
    wkT = _w_T_tiled(Wk)
    wvT = _w_T_tiled(Wv)
    woT = _w_T_tiled(Wo)
    bqT = np.ascontiguousarray(bq.reshape(NDT, P).T).astype(np.float32)
    bkT = np.ascontiguousarray(bk.reshape(NDT, P).T).astype(np.float32)
    bvb = np.ascontiguousarray(np.broadcast_to(bv, (P, D))).astype(np.float32)
    bob = np.ascontiguousarray(np.broadcast_to(bo, (P, D))).astype(np.float32)

    in_maps = []
    for b in range(B):
        kpos = (np.arange(NKT * P).reshape(NKT, P).T).astype(np.int64)  # [P, NKT]
        mask = np.where(kpos < lens[b], 0.0, NEG).astype(np.float32)
        in_maps.append(
            {
                "xqT": _to_bf16_T_tiled(x_Q[b]),
                "xkT": _to_bf16_T_tiled(x_K[b]),
                "xvT": _to_bf16_T_tiled(x_V[b]),
                "wqT": wqT,
                "wkT": wkT,
                "wvT": wvT,
                "woT": woT,
                "bqT": bqT,
                "bkT": bkT,
                "bvb": bvb,
                "bob": bob,
                "mask": np.ascontiguousarray(mask),
            }
        )

    res = run_bass_kernel_spmd(nc, in_maps, core_ids=list(range(B)))
    last_results = res

    out = np.empty((B, S, D), dtype=np.float32)
    for b in range(B):
        o = res.results[b]["out"]  # [P, NST, D]
        out[b] = o.transpose(1, 0, 2).reshape(S, D)
    return out



# revision 51
# speedup vs baseline: 1.2390x; 1.2390x over previous
"""MultiHeadAttention Trainium2 kernel (v2).

Data-parallel: one batch element per NeuronCore (B=8 == n_cores).

Per core:
  - Projections Q/K/V on PE in fp8e4 with DoubleRow perf mode (2 k-tiles of the
    d=512 contraction fused per matmul). Weights pre-scaled by 64 on host to
    escape fp8 subnormals; Q*K recovers the factor in the exp scale, V's factor
    cancels against the ones-column (=64) in the softmax ratio.
  - Q^T/K^T stored fp8 [32-partition, 2-interleave] per head so QK^T also runs
    DoubleRow (d=64 contraction as 32x2).
  - exp on the scores is split across ScalarE (exact, table-based) and
    DVE/GpSimd (Schraudolph: one fused multiply-add rounding into int16 whose
    bits are the bf16 representation of 2^x — exp with one ALU op).
  - Key-padding mask applied by zeroing masked keys' V rows AND ones-column, so
    masked keys drop out of both softmax numerator and denominator exactly; no
    mask bias needed in the exp.
  - P@V with P^T stationary and V' moving: ctx lands in natural [q, h, d]
    layout with the denominator as column 64 -> normalization is a
    per-partition reciprocal + tensor_scalar multiply.
  - ctx transposed via DMA-engine transpose (32x32 xbar tiles) for the output
    projection; out^T = Wo @ ctx^T on PE in bf16; final bias fold on ScalarE.
"""

import numpy as np
import ml_dtypes

import concourse.bass as bass  # noqa: F401
import concourse.tile as tile
from concourse import bacc, mybir
from concourse._compat import get_trn_type
from concourse.bass_utils import run_bass_kernel_spmd

B, S, D = 8, 2048, 512
H, DH = 8, 64
P = 128
NKT = S // P          # 16 key tiles
NQT = S // P          # 16 query tiles
F32 = mybir.dt.float32
BF16 = mybir.dt.bfloat16
F8 = mybir.dt.float8e4
I16 = mybir.dt.int16
MUL = mybir.AluOpType.mult
ADD = mybir.AluOpType.add
DR = mybir.MatmulPerfMode.DoubleRow
Exp = mybir.ActivationFunctionType.Exp
Copy = mybir.ActivationFunctionType.Copy

SW = 64.0             # host-side fp8 weight scale for Wq/Wk/Wv
ONES = 64.0           # ones-column value (= SW so the V scale cancels)
S_EXP = 0.125 / (SW * SW)
SCHR_A = S_EXP * 184.664965        # 128/ln2
SCHR_C = 16250.5                   # 16256 - 5.5 (centers the interp ripple)

# exp tile engine split per (qh, h): 16 key tiles -> A(ScalarE) D(DVE)
# (GpSimd cannot touch PSUM, so Pool is out of the exp path)
EXP_PATTERNS = ("AADADAAADADAADAA",   # A=11, D=5 (even heads)
                "AADADADAADADADAA")   # A=10, D=6 (odd heads)
# engine for Q/K projection psum->sbuf copies: alternate DVE/Pool
PROJ_COPY_ENGINES = ("D", "P")

last_results = None

DEBUG = False
PROJ_ONLY = False


def _eng(nc, tag):
    return {"A": nc.scalar, "D": nc.vector, "P": nc.gpsimd}[tag]


def _build_program(bias_zero: bool):
    nc = bacc.Bacc(get_trn_type() or "TRN2", target_bir_lowering=False)

    xq_d = nc.dram_tensor("xq", (P, 4, S), BF16, kind="ExternalInput")
    xk_d = nc.dram_tensor("xk", (P, 4, S), BF16, kind="ExternalInput")
    xv_d = nc.dram_tensor("xv", (P, 4, S), BF16, kind="ExternalInput")
    wq_d = nc.dram_tensor("wq", (P, 4, D), BF16, kind="ExternalInput")
    wk_d = nc.dram_tensor("wk", (P, 4, D), BF16, kind="ExternalInput")
    wv_d = nc.dram_tensor("wv", (P, 4, D), BF16, kind="ExternalInput")
    wo_d = nc.dram_tensor("wo", (P, 4, D), BF16, kind="ExternalInput")
    maskv_d = nc.dram_tensor("maskv", (P, NKT), F32, kind="ExternalInput")
    masko_d = nc.dram_tensor("masko", (P, NKT, H), F32, kind="ExternalInput")
    bqP_d = nc.dram_tensor("bqP", (P, 2, 2), F32, kind="ExternalInput")
    bkP_d = nc.dram_tensor("bkP", (P, 2, 2), F32, kind="ExternalInput")
    bvB_d = nc.dram_tensor("bvB", (P, D), F32, kind="ExternalInput")
    boP_d = nc.dram_tensor("boP", (P, 4), F32, kind="ExternalInput")
    out_d = nc.dram_tensor("outT", (P, 4, S), BF16, kind="ExternalOutput")
    if DEBUG:
        qdbg_d = nc.dram_tensor("qdbg", (P, 2, 2, S), F8, kind="ExternalOutput")
        kdbg_d = nc.dram_tensor("kdbg", (P, 2, 2, S), F8, kind="ExternalOutput")
        vdbg_d = nc.dram_tensor("vdbg", (P, NKT, H, DH + 1), BF16, kind="ExternalOutput")
        cndbg_d = nc.dram_tensor("cndbg", (P, NQT, D), BF16, kind="ExternalOutput")
        ctdbg_d = nc.dram_tensor("ctdbg", (P, 4, S), BF16, kind="ExternalOutput")
        ptdbg_d = nc.dram_tensor("ptdbg", (P, NKT, 1024), BF16, kind="ExternalOutput")

    with tile.TileContext(nc) as tc:
        with tc.tile_pool(name="persist", bufs=1) as pp:
            # ---- persistent SBUF ----
            wo_sb = pp.tile([P, 4, D], BF16, tag="wo")
            maskv_sb = pp.tile([P, NKT], F32, tag="maskv")
            nc.sync.dma_start(maskv_sb[:], maskv_d[:])
            masko_sb = pp.tile([P, NKT, H], F32, tag="masko")
            nc.sync.dma_start(masko_sb[:], masko_d[:])
            bqP_sb = pp.tile([P, 2, 2], F32, tag="bqP")
            nc.sync.dma_start(bqP_sb[:], bqP_d[:])
            bkP_sb = pp.tile([P, 2, 2], F32, tag="bkP")
            nc.sync.dma_start(bkP_sb[:], bkP_d[:])
            boP_sb = pp.tile([P, 4], F32, tag="boP")
            nc.sync.dma_start(boP_sb[:], boP_d[:])
            if not bias_zero:
                bvB_sb = pp.tile([P, D], F32, tag="bvB")
                nc.sync.dma_start(bvB_sb[:], bvB_d[:])

            qT_sb = pp.tile([P, 2, 2, S], F8, tag="qT")
            kT_sb = pp.tile([P, 2, 2, S], F8, tag="kT")
            v_sb = pp.tile([P, NKT, H, DH + 1], BF16, tag="v")
            ctxn_sb = pp.tile([P, NQT, D], BF16, tag="ctxn")
            ctxT_sb = pp.tile([P, 4, S], BF16, tag="ctxT")

            # ones column = ONES * mask  (zeroed for padded keys)
            nc.gpsimd.tensor_copy(
                v_sb[:, :, :, DH], masko_sb[:, :, :])

            # ---- PSUM pools (6 + 1 + 1 banks) ----
            _scp_cm = tc.tile_pool(name="scp", bufs=3, space="PSUM")
            _cxp_cm = tc.tile_pool(name="cxp", bufs=1, space="PSUM")
            _prp_cm = tc.tile_pool(name="prp", bufs=1, space="PSUM")
            scp = _scp_cm.__enter__()
            cxp = _cxp_cm.__enter__()
            prp = _prp_cm.__enter__()

            # ---- Phase 0: input staging + projections ----
            # load order: K/Q projection operands first (they gate head 0),
            # then V operands (needed mid-head-0), then Wo (needed at qh1)
            _xp_cm = tc.tile_pool(name="xin", bufs=1)
            xp = _xp_cm.__enter__()
            xq_sb = xp.tile([P, 4, S], BF16, tag="xq")
            xk_sb = xp.tile([P, 4, S], BF16, tag="xk")
            xv_sb = xp.tile([P, 4, S], BF16, tag="xv")
            wq_sb = xp.tile([P, 4, D], BF16, tag="wq")
            wk_sb = xp.tile([P, 4, D], BF16, tag="wk")
            wv_sb = xp.tile([P, 4, D], BF16, tag="wv")
            nc.sync.dma_start(wk_sb[:], wk_d[:])
            nc.sync.dma_start(xk_sb[:], xk_d[:])
            nc.sync.dma_start(wq_sb[:], wq_d[:])
            nc.sync.dma_start(xq_sb[:], xq_d[:])
            nc.sync.dma_start(wv_sb[:], wv_d[:])
            nc.sync.dma_start(xv_sb[:], xv_d[:])
            nc.sync.dma_start(wo_sb[:], wo_d[:])

            proj_ct = [0]

            def qk_proj(w_sb, o_sb, x_sb, b_sb, mj, qc):
                # one [128, 512] column block of Q^T or K^T. Computed in bf16
                # (fp8 weights put a systematic error on the scores) and only
                # the OUTPUT is quantized to fp8 for the DoubleRow QK^T.
                mtp, j = mj // 2, mj % 2
                ps = prp.tile([P, 512], F32, tag="pj", name="pj")
                for kt in range(4):
                    nc.tensor.matmul(
                        ps[:],
                        lhsT=w_sb[:, kt, mj * P:(mj + 1) * P],
                        rhs=x_sb[:, kt, qc * 512:(qc + 1) * 512],
                        start=(kt == 0), stop=(kt == 3),
                    )
                nc.vector.tensor_scalar_add(
                    o_sb[:, mtp, j, qc * 512:(qc + 1) * 512],
                    ps[:], b_sb[:, mtp, j:j + 1],
                )

            def v_proj(st):
                # V' rows for s-tile st, natural [s, h, d] with mask zeroing
                # (bf16: fp8 Wv quantization passes straight through to ctx)
                ps = prp.tile([P, 512], F32, tag="pj", name="pv")
                for kt in range(4):
                    nc.tensor.matmul(
                        ps[:],
                        lhsT=xv_sb[:, kt, st * P:(st + 1) * P],
                        rhs=wv_sb[:, kt, :],
                        start=(kt == 0), stop=(kt == 3),
                    )
                vout = v_sb[:, st, :, 0:DH]
                if bias_zero:
                    nc.vector.tensor_scalar(
                        vout, ps[:].rearrange("p (h d) -> p h d", h=H),
                        maskv_sb[:, st:st + 1], None, MUL,
                    )
                else:
                    tmp = xp.tile([P, 512], F32, tag="vtmp")
                    nc.vector.tensor_tensor(
                        out=tmp[:], in0=ps[:], in1=bvB_sb[:], op=ADD)
                    nc.vector.tensor_scalar(
                        vout, tmp[:].rearrange("p (h d) -> p h d", h=H),
                        maskv_sb[:, st:st + 1], None, MUL,
                    )

            # projections needed to start head 0 (mt'=0 K cols, qh0 Q cols);
            # the rest is woven into the first heads' phase-1 steps below
            for mj in range(2):
                for qc in range(4):
                    qk_proj(wk_sb, kT_sb, xk_sb, bkP_sb, mj, qc)
            for mj in range(2):
                for qc in range(2):
                    qk_proj(wq_sb, qT_sb, xq_sb, bqP_sb, mj, qc)

            if PROJ_ONLY:
                for mj in (2, 3):
                    for qc in range(4):
                        qk_proj(wk_sb, kT_sb, xk_sb, bkP_sb, mj, qc)
                for mj in range(4):
                    for qc in range(4):
                        if (mj, qc) not in [(m, q) for m in (0, 1) for q in (0, 1)]:
                            qk_proj(wq_sb, qT_sb, xq_sb, bqP_sb, mj, qc)
                for st in range(NKT):
                    v_proj(st)

            def _mk_v(st):
                return lambda: v_proj(st)

            def _mk_qk(w, o, x, b, mj, qc):
                return lambda: qk_proj(w, o, x, b, mj, qc)

            weave = {
                0: [_mk_v(st) for st in range(NKT)],
                1: [_mk_qk(wk_sb, kT_sb, xk_sb, bkP_sb, mj, qc)
                    for mj in (2, 3) for qc in range(4)]
                   + [_mk_qk(wq_sb, qT_sb, xq_sb, bqP_sb, mj, qc)
                      for mj in (2, 3) for qc in range(2)],
                2: [_mk_qk(wq_sb, qT_sb, xq_sb, bqP_sb, mj, qc)
                    for mj in range(4) for qc in (2, 3)],
            }

            # ---- attention: for qh (1024-query half), per head ----
            with tc.tile_pool(name="ptp", bufs=34) as ptp, \
                 tc.tile_pool(name="rcp", bufs=4) as rcp, \
                 tc.tile_pool(name="obp", bufs=4) as obp:

                def oproj(mo, qc):
                    # out^T block [128 dout, 512 q] (bf16)
                    pso = prp.tile([P, 512], F32, tag="pj", name="po")
                    for kt in range(4):
                        nc.tensor.matmul(
                            pso[:],
                            lhsT=wo_sb[:, kt, mo * P:(mo + 1) * P],
                            rhs=ctxT_sb[:, kt, qc * 512:(qc + 1) * 512],
                            start=(kt == 0), stop=(kt == 3),
                        )
                    ot = obp.tile([P, 512], BF16, tag="ot")
                    nc.vector.tensor_scalar_add(
                        ot[:], pso[:], boP_sb[:, mo:mo + 1])
                    nc.sync.dma_start(
                        out_d[:, mo, qc * 512:(qc + 1) * 512], ot[:])

                oprojq = []

                def phase2_gen(qh, h, pts):
                    # P@V per query tile (P^T stationary, V moving), yielded in
                    # chunks so it interleaves with the next head's QK/exp.
                    # Each query tile's 16-matmul accumulation chain runs
                    # UNINTERLEAVED (psum accumulation groups cannot nest).
                    for qp in range(2):
                        cxq = cxp.tile([P, 4, DH + 1], F32, tag="cx", name="cx")
                        for qt in range(4):
                            for t in range(NKT):
                                nc.tensor.matmul(
                                    cxq[:, qt, :],
                                    lhsT=pts[t][:, qp * 512 + qt * P:
                                                qp * 512 + (qt + 1) * P],
                                    rhs=v_sb[:, t, h, :],
                                    start=(t == 0), stop=(t == NKT - 1),
                                )
                            yield
                        rc = rcp.tile([P, 4], F32, tag="rc")
                        nc.vector.reciprocal(
                            rc[:], cxq[:, :, DH:DH + 1].rearrange("p a o -> p (a o)"))
                        for qt in range(4):
                            gqt = qh * 8 + qp * 4 + qt
                            nc.vector.tensor_scalar(
                                ctxn_sb[:, gqt, h * DH:(h + 1) * DH],
                                cxq[:, qt, 0:DH], rc[:, qt:qt + 1], None, MUL,
                            )
                        yield

                def drain(gen):
                    if gen is not None:
                        for _ in gen:
                            pass

                pend = None
                for qh in range(0 if not PROJ_ONLY else 2, 2):
                    q0 = qh * 1024
                    for h in range(H):
                        hh = qh * 8 + h
                        mtp, hp = h // 4, (h % 4) * 32
                        # phase 1: QK^T + exp for all 16 key tiles of (qh, h),
                        # with the previous head's P@V interleaved
                        pts = []
                        for t in range(NKT):
                            if weave.get(hh):
                                weave[hh].pop(0)()
                            if oprojq and t % 2 == 1:
                                oproj(*oprojq.pop(0))
                            sc = scp.tile([P, 2, 512], F32, tag="sc", name="sc")
                            for qc in range(2):
                                nc.tensor.matmul(
                                    sc[:, qc, :],
                                    lhsT=kT_sb[hp:hp + 32, mtp, :, t * P:(t + 1) * P],
                                    rhs=qT_sb[hp:hp + 32, mtp, :,
                                              q0 + qc * 512:q0 + (qc + 1) * 512],
                                    start=True, stop=True, perf_mode=DR,
                                    tile_position=(hp, 0),
                                )
                            pt = ptp.tile([P, 1024], BF16, tag="pt")
                            etag = EXP_PATTERNS[h % 2][t]
                            scf = sc[:].rearrange("p a b -> p (a b)")
                            if etag == "A":
                                nc.scalar.activation(
                                    pt[:], scf, Exp, bias=0.0, scale=S_EXP)
                            else:
                                _eng(nc, etag).tensor_scalar(
                                    pt[:].bitcast(I16), scf,
                                    SCHR_A, SCHR_C, MUL, ADD)
                            if DEBUG and qh == 0 and h == 0:
                                nc.sync.dma_start(ptdbg_d[:, t, :], pt[:])
                            pts.append(pt)
                            if pend is not None:
                                for _ in range(3):
                                    if next(pend, StopIteration) is StopIteration:
                                        pend = None
                                        break
                        drain(pend)
                        pend = phase2_gen(qh, h, pts)
                    # end of qh: finish the last head, then transpose ctx rows
                    drain(pend)
                    pend = None
                    for qt in range(8):
                        gqt = qh * 8 + qt
                        nc.sync.dma_start_transpose(
                            ctxT_sb[:, :, gqt * P:(gqt + 1) * P],
                            ctxn_sb[:, gqt, :],
                        )
                    if qh == 0:
                        oprojq = [(mo, qc) for mo in range(4) for qc in range(2)]
                    else:
                        rest = [(mo, qc) for mo in range(4) for qc in range(2, 4)]
                        for mo, qc in oprojq + rest:
                            oproj(mo, qc)
                        oprojq = []

            if PROJ_ONLY:
                nc.sync.dma_start(
                    out_d[:].rearrange("p m (a h c) -> p (m a) h c", a=4, h=H),
                    v_sb[:, :, :, 0:DH])
            if DEBUG:
                nc.sync.dma_start(qdbg_d[:], qT_sb[:])
                nc.sync.dma_start(kdbg_d[:], kT_sb[:])
                nc.sync.dma_start(vdbg_d[:], v_sb[:])
                if not PROJ_ONLY:
                    nc.sync.dma_start(cndbg_d[:], ctxn_sb[:])
                    nc.sync.dma_start(ctdbg_d[:], ctxT_sb[:])

            _xp_cm.__exit__(None, None, None)
            _prp_cm.__exit__(None, None, None)
            _cxp_cm.__exit__(None, None, None)
            _scp_cm.__exit__(None, None, None)

    nc.compile()
    return nc


_program_cache = {}


def _get_program(bias_zero: bool = True):
    if bias_zero not in _program_cache:
        _program_cache[bias_zero] = _build_program(bias_zero)
    return _program_cache[bias_zero]


# ---- host-side packing ----

def _qk_out_perm():
    # column c = mt'*256 + j*128 + r  ->  output neuron (head, dim)
    c = np.arange(D)
    mtp, rem = c // 256, c % 256
    j, r = rem // 128, rem % 128
    head = mtp * 4 + r // 32
    dim = j * 32 + (r % 32)
    return head * DH + dim  # o[c]


def _pack_w_qk(w, perm):
    # torch Linear weight [out, in] -> scaled W^T [in, out] with permuted
    # cols, bf16, plain in-dim tiling [128, 4, out]
    wt = np.ascontiguousarray(w.T * SW)[:, perm]  # [in, out_cols]
    arr = wt.reshape(4, P, D).transpose(1, 0, 2)
    return np.ascontiguousarray(arr.astype(ml_dtypes.bfloat16))


def _pack_w_v(w):
    # bf16, scaled by SW, plain [128, 4, D] (in-dim tiled)
    wt = np.ascontiguousarray(w.T * SW)
    arr = wt.reshape(4, P, D).transpose(1, 0, 2)
    return np.ascontiguousarray(arr.astype(ml_dtypes.bfloat16))


def _pack_xv(x):
    xt = np.ascontiguousarray(x.T)  # [D, S]
    arr = xt.reshape(4, P, S).transpose(1, 0, 2)
    return np.ascontiguousarray(arr.astype(ml_dtypes.bfloat16))


def _pack_wo(w):
    # out^T = Wo @ ctx^T: lhsT = Wo^T [hd, dout] tiles [128, 4kt, 512]
    wt = np.ascontiguousarray(w.T)  # [hd 512, dout 512]
    arr = wt.reshape(4, P, D).transpose(1, 0, 2)
    return np.ascontiguousarray(arr.astype(ml_dtypes.bfloat16))


def kernel(**inputs):
    global last_results
    x_Q = np.asarray(inputs["x_Q"], dtype=np.float32)
    x_K = np.asarray(inputs["x_K"], dtype=np.float32)
    x_V = np.asarray(inputs["x_V"], dtype=np.float32)
    Wq = np.asarray(inputs["Wq"], dtype=np.float32)
    Wk = np.asarray(inputs["Wk"], dtype=np.float32)
    Wv = np.asarray(inputs["Wv"], dtype=np.float32)
    Wo = np.asarray(inputs["Wo"], dtype=np.float32)
    bq = np.asarray(inputs["bq"], dtype=np.float32)
    bk = np.asarray(inputs["bk"], dtype=np.float32)
    bv = np.asarray(inputs["bv"], dtype=np.float32)
    bo = np.asarray(inputs["bo"], dtype=np.float32)
    lens = np.asarray(inputs["src_batch_lens"]).astype(np.int64)

    bias_zero = not (bv.any())
    nc = _get_program(bias_zero)

    perm = _qk_out_perm()
    wq = _pack_w_qk(Wq, perm)
    wk = _pack_w_qk(Wk, perm)
    wv = _pack_w_v(Wv)
    wo = _pack_wo(Wo)
    # permuted, scaled per-(mt', j) biases [128, 2, 2]
    bqP = (SW * bq[perm]).reshape(2, 2, P).transpose(2, 0, 1).astype(np.float32)
    bkP = (SW * bk[perm]).reshape(2, 2, P).transpose(2, 0, 1).astype(np.float32)
    bqP = np.ascontiguousarray(bqP)
    bkP = np.ascontiguousarray(bkP)
    bvB = np.ascontiguousarray(np.broadcast_to(SW * bv, (P, D))).astype(np.float32)
    boP = np.ascontiguousarray(bo.reshape(4, P).T).astype(np.float32)

    kpos = np.arange(NKT * P).reshape(NKT, P).T  # [P, NKT]
    in_maps = []
    for b in range(B):
        m = (kpos < lens[b]).astype(np.float32)
        maskv = np.ascontiguousarray(m)
        masko = np.ascontiguousarray(
            np.repeat((ONES * m)[:, :, None], H, axis=2)).astype(np.float32)
        in_maps.append({
            "xq": _pack_xv(x_Q[b]),
            "xk": _pack_xv(x_K[b]),
            "xv": _pack_xv(x_V[b]),
            "wq": wq, "wk": wk, "wv": wv, "wo": wo,
            "maskv": maskv, "masko": masko,
            "bqP": bqP, "bkP": bkP, "bvB": bvB, "boP": boP,
        })

    res = run_bass_kernel_spmd(nc, in_maps, core_ids=list(range(B)))
    last_results = res

    out = np.empty((B, S, D), dtype=np.float32)
    for b in range(B):
        oT = res.results[b]["outT"].astype(np.float32)  # [128, 4, S]
        out[b] = oT.transpose(2, 1, 0).reshape(S, D)
    return out


if __name__ == "__main__":
    from concourse.timeline_sim import TimelineSim
    prog = _get_program(True)
    ts = TimelineSim(prog, no_exec=True, trace=False)
    print(f"TimelineSim: {ts.simulate():.0f} ns")


# revision 54
# speedup vs baseline: 1.2545x; 1.0125x over previous
"""MultiHeadAttention Trainium2 kernel (v2).

Data-parallel: one batch element per NeuronCore (B=8 == n_cores).

Per core:
  - Projections Q/K/V on PE in fp8e4 with DoubleRow perf mode (2 k-tiles of the
    d=512 contraction fused per matmul). Weights pre-scaled by 64 on host to
    escape fp8 subnormals; Q*K recovers the factor in the exp scale, V's factor
    cancels against the ones-column (=64) in the softmax ratio.
  - Q^T/K^T stored fp8 [32-partition, 2-interleave] per head so QK^T also runs
    DoubleRow (d=64 contraction as 32x2).
  - exp on the scores is split across ScalarE (exact, table-based) and
    DVE/GpSimd (Schraudolph: one fused multiply-add rounding into int16 whose
    bits are the bf16 representation of 2^x — exp with one ALU op).
  - Key-padding mask applied by zeroing masked keys' V rows AND ones-column, so
    masked keys drop out of both softmax numerator and denominator exactly; no
    mask bias needed in the exp.
  - P@V with P^T stationary and V' moving: ctx lands in natural [q, h, d]
    layout with the denominator as column 64 -> normalization is a
    per-partition reciprocal + tensor_scalar multiply.
  - ctx transposed via DMA-engine transpose (32x32 xbar tiles) for the output
    projection; out^T = Wo @ ctx^T on PE in bf16; final bias fold on ScalarE.
"""

import numpy as np
import ml_dtypes

import concourse.bass as bass  # noqa: F401
import concourse.tile as tile
from concourse import bacc, mybir
from concourse._compat import get_trn_type
from concourse.bass_utils import run_bass_kernel_spmd

B, S, D = 8, 2048, 512
H, DH = 8, 64
P = 128
NKT = S // P          # 16 key tiles
NQT = S // P          # 16 query tiles
F32 = mybir.dt.float32
BF16 = mybir.dt.bfloat16
F8 = mybir.dt.float8e4
I16 = mybir.dt.int16
MUL = mybir.AluOpType.mult
ADD = mybir.AluOpType.add
DR = mybir.MatmulPerfMode.DoubleRow
Exp = mybir.ActivationFunctionType.Exp
Copy = mybir.ActivationFunctionType.Copy

SW = 64.0             # host-side fp8 weight scale for Wq/Wk/Wv
ONES = 64.0           # ones-column value (= SW so the V scale cancels)
S_EXP = 0.125 / (SW * SW)
SCHR_A = S_EXP * 184.664965        # 128/ln2
SCHR_C = 16250.5                   # 16256 - 5.5 (centers the interp ripple)

# exp tile engine split per (qh, h): 16 key tiles -> A(ScalarE) D(DVE)
# (GpSimd cannot touch PSUM, so Pool is out of the exp path)
EXP_PATTERNS = ("AADADAAADADAADAA",   # A=11, D=5 (even heads)
                "AADADADAADADADAA")   # A=10, D=6 (odd heads)
# engine for Q/K projection psum->sbuf copies: alternate DVE/Pool
PROJ_COPY_ENGINES = ("D", "P")

last_results = None

DEBUG = False
PROJ_ONLY = False


def _eng(nc, tag):
    return {"A": nc.scalar, "D": nc.vector, "P": nc.gpsimd}[tag]


def _build_program(bias_zero: bool):
    nc = bacc.Bacc(get_trn_type() or "TRN2", target_bir_lowering=False)

    xq_d = nc.dram_tensor("xq", (P, 4, S), BF16, kind="ExternalInput")
    xk_d = nc.dram_tensor("xk", (P, 4, S), BF16, kind="ExternalInput")
    xv_d = nc.dram_tensor("xv", (P, 4, S), BF16, kind="ExternalInput")
    wq_d = nc.dram_tensor("wq", (P, 4, D), BF16, kind="ExternalInput")
    wk_d = nc.dram_tensor("wk", (P, 4, D), BF16, kind="ExternalInput")
    wv_d = nc.dram_tensor("wv", (P, 4, D), BF16, kind="ExternalInput")
    wo_d = nc.dram_tensor("wo", (P, 4, D), BF16, kind="ExternalInput")
    maskv_d = nc.dram_tensor("maskv", (P, NKT), F32, kind="ExternalInput")
    masko_d = nc.dram_tensor("masko", (P, NKT, H), F32, kind="ExternalInput")
    bqP_d = nc.dram_tensor("bqP", (P, 2, 2), F32, kind="ExternalInput")
    bkP_d = nc.dram_tensor("bkP", (P, 2, 2), F32, kind="ExternalInput")
    bvB_d = nc.dram_tensor("bvB", (P, D), F32, kind="ExternalInput")
    boP_d = nc.dram_tensor("boP", (P, 4), F32, kind="ExternalInput")
    out_d = nc.dram_tensor("outT", (P, 4, S), BF16, kind="ExternalOutput")
    if DEBUG:
        qdbg_d = nc.dram_tensor("qdbg", (P, 2, 2, S), F8, kind="ExternalOutput")
        kdbg_d = nc.dram_tensor("kdbg", (P, 2, 2, S), F8, kind="ExternalOutput")
        vdbg_d = nc.dram_tensor("vdbg", (P, NKT, H, DH + 1), BF16, kind="ExternalOutput")
        cndbg_d = nc.dram_tensor("cndbg", (P, NQT, D), BF16, kind="ExternalOutput")
        ctdbg_d = nc.dram_tensor("ctdbg", (P, 4, S), BF16, kind="ExternalOutput")
        ptdbg_d = nc.dram_tensor("ptdbg", (P, NKT, 1024), BF16, kind="ExternalOutput")

    with tile.TileContext(nc) as tc:
        with tc.tile_pool(name="persist", bufs=1) as pp:
            # ---- persistent SBUF ----
            wo_sb = pp.tile([P, 4, D], BF16, tag="wo")
            maskv_sb = pp.tile([P, NKT], F32, tag="maskv")
            nc.sync.dma_start(maskv_sb[:], maskv_d[:])
            masko_sb = pp.tile([P, NKT, H], F32, tag="masko")
            nc.sync.dma_start(masko_sb[:], masko_d[:])
            bqP_sb = pp.tile([P, 2, 2], F32, tag="bqP")
            nc.sync.dma_start(bqP_sb[:], bqP_d[:])
            bkP_sb = pp.tile([P, 2, 2], F32, tag="bkP")
            nc.sync.dma_start(bkP_sb[:], bkP_d[:])
            boP_sb = pp.tile([P, 4], F32, tag="boP")
            nc.sync.dma_start(boP_sb[:], boP_d[:])
            if not bias_zero:
                bvB_sb = pp.tile([P, D], F32, tag="bvB")
                nc.sync.dma_start(bvB_sb[:], bvB_d[:])

            qT_sb = pp.tile([P, 2, 2, S], F8, tag="qT")
            kT_sb = pp.tile([P, 2, 2, S], F8, tag="kT")
            v_sb = pp.tile([P, NKT, H, DH + 1], BF16, tag="v")
            ctxn_sb = pp.tile([P, NQT, D], BF16, tag="ctxn")
            ctxT_sb = pp.tile([P, 4, S], BF16, tag="ctxT")

            # ones column = ONES * mask  (zeroed for padded keys)
            nc.gpsimd.tensor_copy(
                v_sb[:, :, :, DH], masko_sb[:, :, :])

            # ---- PSUM pools (6 + 1 + 1 banks) ----
            _scp_cm = tc.tile_pool(name="scp", bufs=3, space="PSUM")
            _cxp_cm = tc.tile_pool(name="cxp", bufs=1, space="PSUM")
            _prp_cm = tc.tile_pool(name="prp", bufs=1, space="PSUM")
            scp = _scp_cm.__enter__()
            cxp = _cxp_cm.__enter__()
            prp = _prp_cm.__enter__()

            # ---- Phase 0: input staging + projections ----
            # load order: K/Q projection operands first (they gate head 0),
            # then V operands (needed mid-head-0), then Wo (needed at qh1)
            _xp_cm = tc.tile_pool(name="xin", bufs=1)
            xp = _xp_cm.__enter__()
            xq_sb = xp.tile([P, 4, S], BF16, tag="xq")
            xk_sb = xp.tile([P, 4, S], BF16, tag="xk")
            xv_sb = xp.tile([P, 4, S], BF16, tag="xv")
            wq_sb = xp.tile([P, 4, D], BF16, tag="wq")
            wk_sb = xp.tile([P, 4, D], BF16, tag="wk")
            wv_sb = xp.tile([P, 4, D], BF16, tag="wv")
            nc.sync.dma_start(wk_sb[:], wk_d[:])
            nc.sync.dma_start(xk_sb[:], xk_d[:])
            nc.sync.dma_start(wq_sb[:], wq_d[:])
            nc.sync.dma_start(xq_sb[:], xq_d[:])
            nc.sync.dma_start(wv_sb[:], wv_d[:])
            nc.sync.dma_start(xv_sb[:], xv_d[:])
            nc.sync.dma_start(wo_sb[:], wo_d[:])

            proj_ct = [0]

            def qk_proj(w_sb, o_sb, x_sb, b_sb, mj, qc, upfront=False):
                # one [128, 512] column block of Q^T or K^T. Computed in bf16
                # (fp8 weights put a systematic error on the scores) and only
                # the OUTPUT is quantized to fp8 for the DoubleRow QK^T.
                # Upfront groups borrow the (idle) 3-deep score pool so the
                # chains pipeline instead of serializing through prp's 1 slot.
                mtp, j = mj // 2, mj % 2
                if upfront:
                    ps = scp.tile([P, 2, 512], F32, tag="sc", name="pj")[:, 0, :]
                else:
                    ps = prp.tile([P, 512], F32, tag="pj", name="pj")[:]
                for kt in range(4):
                    nc.tensor.matmul(
                        ps,
                        lhsT=w_sb[:, kt, mj * P:(mj + 1) * P],
                        rhs=x_sb[:, kt, qc * 512:(qc + 1) * 512],
                        start=(kt == 0), stop=(kt == 3),
                    )
                nc.vector.tensor_scalar_add(
                    o_sb[:, mtp, j, qc * 512:(qc + 1) * 512],
                    ps, b_sb[:, mtp, j:j + 1],
                )

            def v_proj(st):
                # V' rows for s-tile st, natural [s, h, d] with mask zeroing
                # (bf16: fp8 Wv quantization passes straight through to ctx)
                ps = prp.tile([P, 512], F32, tag="pj", name="pv")
                for kt in range(4):
                    nc.tensor.matmul(
                        ps[:],
                        lhsT=xv_sb[:, kt, st * P:(st + 1) * P],
                        rhs=wv_sb[:, kt, :],
                        start=(kt == 0), stop=(kt == 3),
                    )
                vout = v_sb[:, st, :, 0:DH]
                if bias_zero:
                    nc.vector.tensor_scalar(
                        vout, ps[:].rearrange("p (h d) -> p h d", h=H),
                        maskv_sb[:, st:st + 1], None, MUL,
                    )
                else:
                    tmp = xp.tile([P, 512], F32, tag="vtmp")
                    nc.vector.tensor_tensor(
                        out=tmp[:], in0=ps[:], in1=bvB_sb[:], op=ADD)
                    nc.vector.tensor_scalar(
                        vout, tmp[:].rearrange("p (h d) -> p h d", h=H),
                        maskv_sb[:, st:st + 1], None, MUL,
                    )

            # projections needed to start head 0 (mt'=0 K cols, qh0 Q cols);
            # the rest is woven into the first heads' phase-1 steps below
            for mj in range(2):
                for qc in range(4):
                    qk_proj(wk_sb, kT_sb, xk_sb, bkP_sb, mj, qc, upfront=True)
            for mj in range(2):
                for qc in range(2):
                    qk_proj(wq_sb, qT_sb, xq_sb, bqP_sb, mj, qc, upfront=True)

            if PROJ_ONLY:
                for mj in (2, 3):
                    for qc in range(4):
                        qk_proj(wk_sb, kT_sb, xk_sb, bkP_sb, mj, qc)
                for mj in range(4):
                    for qc in range(4):
                        if (mj, qc) not in [(m, q) for m in (0, 1) for q in (0, 1)]:
                            qk_proj(wq_sb, qT_sb, xq_sb, bqP_sb, mj, qc)
                for st in range(NKT):
                    v_proj(st)

            def _mk_v(st):
                return lambda: v_proj(st)

            def _mk_qk(w, o, x, b, mj, qc):
                return lambda: qk_proj(w, o, x, b, mj, qc)

            weave = {
                0: [_mk_v(st) for st in range(NKT)],
                1: [_mk_qk(wk_sb, kT_sb, xk_sb, bkP_sb, mj, qc)
                    for mj in (2, 3) for qc in range(4)]
                   + [_mk_qk(wq_sb, qT_sb, xq_sb, bqP_sb, mj, qc)
                      for mj in (2, 3) for qc in range(2)],
                2: [_mk_qk(wq_sb, qT_sb, xq_sb, bqP_sb, mj, qc)
                    for mj in range(4) for qc in (2, 3)],
            }

            # ---- attention: for qh (1024-query half), per head ----
            with tc.tile_pool(name="ptp", bufs=34) as ptp, \
                 tc.tile_pool(name="rcp", bufs=4) as rcp, \
                 tc.tile_pool(name="obp", bufs=4) as obp:

                def oproj(mo, qc):
                    # out^T block [128 dout, 512 q] (bf16)
                    pso = prp.tile([P, 512], F32, tag="pj", name="po")
                    for kt in range(4):
                        nc.tensor.matmul(
                            pso[:],
                            lhsT=wo_sb[:, kt, mo * P:(mo + 1) * P],
                            rhs=ctxT_sb[:, kt, qc * 512:(qc + 1) * 512],
                            start=(kt == 0), stop=(kt == 3),
                        )
                    ot = obp.tile([P, 512], BF16, tag="ot")
                    nc.vector.tensor_scalar_add(
                        ot[:], pso[:], boP_sb[:, mo:mo + 1])
                    nc.sync.dma_start(
                        out_d[:, mo, qc * 512:(qc + 1) * 512], ot[:])

                oprojq = []

                def phase2_gen(qh, h, pts):
                    # P@V per query tile (P^T stationary, V moving), yielded in
                    # chunks so it interleaves with the next head's QK/exp.
                    # Each query tile's 16-matmul accumulation chain runs
                    # UNINTERLEAVED (psum accumulation groups cannot nest).
                    for qp in range(2):
                        cxq = cxp.tile([P, 4, DH + 1], F32, tag="cx", name="cx")
                        for qt in range(4):
                            for t in range(NKT):
                                nc.tensor.matmul(
                                    cxq[:, qt, :],
                                    lhsT=pts[t][:, qp * 512 + qt * P:
                                                qp * 512 + (qt + 1) * P],
                                    rhs=v_sb[:, t, h, :],
                                    start=(t == 0), stop=(t == NKT - 1),
                                )
                            yield
                        rc = rcp.tile([P, 4], F32, tag="rc")
                        nc.vector.reciprocal(
                            rc[:], cxq[:, :, DH:DH + 1].rearrange("p a o -> p (a o)"))
                        for qt in range(4):
                            gqt = qh * 8 + qp * 4 + qt
                            nc.vector.tensor_scalar(
                                ctxn_sb[:, gqt, h * DH:(h + 1) * DH],
                                cxq[:, qt, 0:DH], rc[:, qt:qt + 1], None, MUL,
                            )
                        yield

                def drain(gen):
                    if gen is not None:
                        for _ in gen:
                            pass

                pend = None
                for qh in range(0 if not PROJ_ONLY else 2, 2):
                    q0 = qh * 1024
                    for h in range(H):
                        hh = qh * 8 + h
                        mtp, hp = h // 4, (h % 4) * 32
                        # phase 1: QK^T + exp for all 16 key tiles of (qh, h),
                        # with the previous head's P@V interleaved
                        pts = []
                        for t in range(NKT):
                            if weave.get(hh):
                                weave[hh].pop(0)()
                            if oprojq and t % 2 == 1:
                                oproj(*oprojq.pop(0))
                            sc = scp.tile([P, 2, 512], F32, tag="sc", name="sc")
                            for qc in range(2):
                                nc.tensor.matmul(
                                    sc[:, qc, :],
                                    lhsT=kT_sb[hp:hp + 32, mtp, :, t * P:(t + 1) * P],
                                    rhs=qT_sb[hp:hp + 32, mtp, :,
                                              q0 + qc * 512:q0 + (qc + 1) * 512],
                                    start=True, stop=True, perf_mode=DR,
                                    tile_position=(hp, 0),
                                )
                            pt = ptp.tile([P, 1024], BF16, tag="pt")
                            etag = EXP_PATTERNS[h % 2][t]
                            scf = sc[:].rearrange("p a b -> p (a b)")
                            if etag == "A":
                                nc.scalar.activation(
                                    pt[:], scf, Exp, bias=0.0, scale=S_EXP)
                            else:
                                _eng(nc, etag).tensor_scalar(
                                    pt[:].bitcast(I16), scf,
                                    SCHR_A, SCHR_C, MUL, ADD)
                            if DEBUG and qh == 0 and h == 0:
                                nc.sync.dma_start(ptdbg_d[:, t, :], pt[:])
                            pts.append(pt)
                            if pend is not None:
                                for _ in range(3):
                                    if next(pend, StopIteration) is StopIteration:
                                        pend = None
                                        break
                        drain(pend)
                        pend = phase2_gen(qh, h, pts)
                    # end of qh: finish the last head, then transpose ctx rows
                    drain(pend)
                    pend = None
                    for qt in range(8):
                        gqt = qh * 8 + qt
                        nc.sync.dma_start_transpose(
                            ctxT_sb[:, :, gqt * P:(gqt + 1) * P],
                            ctxn_sb[:, gqt, :],
                        )
                    if qh == 0:
                        oprojq = [(mo, qc) for mo in range(4) for qc in range(2)]
                    else:
                        rest = [(mo, qc) for mo in range(4) for qc in range(2, 4)]
                        for mo, qc in oprojq + rest:
                            oproj(mo, qc)
                        oprojq = []

            if PROJ_ONLY:
                nc.sync.dma_start(
                    out_d[:].rearrange("p m (a h c) -> p (m a) h c", a=4, h=H),
                    v_sb[:, :, :, 0:DH])
            if DEBUG:
                nc.sync.dma_start(qdbg_d[:], qT_sb[:])
                nc.sync.dma_start(kdbg_d[:], kT_sb[:])
                nc.sync.dma_start(vdbg_d[:], v_sb[:])
                if not PROJ_ONLY:
                    nc.sync.dma_start(cndbg_d[:], ctxn_sb[:])
                    nc.sync.dma_start(ctdbg_d[:], ctxT_sb[:])

            _xp_cm.__exit__(None, None, None)
            _prp_cm.__exit__(None, None, None)
            _cxp_cm.__exit__(None, None, None)
            _scp_cm.__exit__(None, None, None)

    nc.compile()
    return nc


_program_cache = {}


def _get_program(bias_zero: bool = True):
    if bias_zero not in _program_cache:
        _program_cache[bias_zero] = _build_program(bias_zero)
    return _program_cache[bias_zero]


# ---- host-side packing ----

def _qk_out_perm():
    # column c = mt'*256 + j*128 + r  ->  output neuron (head, dim)
    c = np.arange(D)
    mtp, rem = c // 256, c % 256
    j, r = rem // 128, rem % 128
    head = mtp * 4 + r // 32
    dim = j * 32 + (r % 32)
    return head * DH + dim  # o[c]


def _pack_w_qk(w, perm):
    # torch Linear weight [out, in] -> scaled W^T [in, out] with permuted
    # cols, bf16, plain in-dim tiling [128, 4, out]
    wt = np.ascontiguousarray(w.T * SW)[:, perm]  # [in, out_cols]
    arr = wt.reshape(4, P, D).transpose(1, 0, 2)
    return np.ascontiguousarray(arr.astype(ml_dtypes.bfloat16))


def _pack_w_v(w):
    # bf16, scaled by SW, plain [128, 4, D] (in-dim tiled)
    wt = np.ascontiguousarray(w.T * SW)
    arr = wt.reshape(4, P, D).transpose(1, 0, 2)
    return np.ascontiguousarray(arr.astype(ml_dtypes.bfloat16))


def _pack_xv(x):
    xt = np.ascontiguousarray(x.T)  # [D, S]
    arr = xt.reshape(4, P, S).transpose(1, 0, 2)
    return np.ascontiguousarray(arr.astype(ml_dtypes.bfloat16))


def _pack_wo(w):
    # out^T = Wo @ ctx^T: lhsT = Wo^T [hd, dout] tiles [128, 4kt, 512]
    wt = np.ascontiguousarray(w.T)  # [hd 512, dout 512]
    arr = wt.reshape(4, P, D).transpose(1, 0, 2)
    return np.ascontiguousarray(arr.astype(ml_dtypes.bfloat16))


def kernel(**inputs):
    global last_results
    x_Q = np.asarray(inputs["x_Q"], dtype=np.float32)
    x_K = np.asarray(inputs["x_K"], dtype=np.float32)
    x_V = np.asarray(inputs["x_V"], dtype=np.float32)
    Wq = np.asarray(inputs["Wq"], dtype=np.float32)
    Wk = np.asarray(inputs["Wk"], dtype=np.float32)
    Wv = np.asarray(inputs["Wv"], dtype=np.float32)
    Wo = np.asarray(inputs["Wo"], dtype=np.float32)
    bq = np.asarray(inputs["bq"], dtype=np.float32)
    bk = np.asarray(inputs["bk"], dtype=np.float32)
    bv = np.asarray(inputs["bv"], dtype=np.float32)
    bo = np.asarray(inputs["bo"], dtype=np.float32)
    lens = np.asarray(inputs["src_batch_lens"]).astype(np.int64)

    bias_zero = not (bv.any())
    nc = _get_program(bias_zero)

    perm = _qk_out_perm()
    wq = _pack_w_qk(Wq, perm)
    wk = _pack_w_qk(Wk, perm)
    wv = _pack_w_v(Wv)
    wo = _pack_wo(Wo)
    # permuted, scaled per-(mt', j) biases [128, 2, 2]
    bqP = (SW * bq[perm]).reshape(2, 2, P).transpose(2, 0, 1).astype(np.float32)
    bkP = (SW * bk[perm]).reshape(2, 2, P).transpose(2, 0, 1).astype(np.float32)
    bqP = np.ascontiguousarray(bqP)
    bkP = np.ascontiguousarray(bkP)
    bvB = np.ascontiguousarray(np.broadcast_to(SW * bv, (P, D))).astype(np.float32)
    boP = np.ascontiguousarray(bo.reshape(4, P).T).astype(np.float32)

    kpos = np.arange(NKT * P).reshape(NKT, P).T  # [P, NKT]
    in_maps = []
    for b in range(B):
        m = (kpos < lens[b]).astype(np.float32)
        maskv = np.ascontiguousarray(m)
        masko = np.ascontiguousarray(
            np.repeat((ONES * m)[:, :, None], H, axis=2)).astype(np.float32)
        in_maps.append({
            "xq": _pack_xv(x_Q[b]),
            "xk": _pack_xv(x_K[b]),
            "xv": _pack_xv(x_V[b]),
            "wq": wq, "wk": wk, "wv": wv, "wo": wo,
            "maskv": maskv, "masko": masko,
            "bqP": bqP, "bkP": bkP, "bvB": bvB, "boP": boP,
        })

    res = run_bass_kernel_spmd(nc, in_maps, core_ids=list(range(B)))
    last_results = res

    out = np.empty((B, S, D), dtype=np.float32)
    for b in range(B):
        oT = res.results[b]["outT"].astype(np.float32)  # [128, 4, S]
        out[b] = oT.transpose(2, 1, 0).reshape(S, D)
    return out


if __name__ == "__main__":
    from concourse.timeline_sim import TimelineSim
    prog = _get_program(True)
    ts = TimelineSim(prog, no_exec=True, trace=False)
    print(f"TimelineSim: {ts.simulate():.0f} ns")
